# revision 29
# baseline (speedup 1.0000x reference)
"""Trainium2 Bass kernel for nn_K_attention_ex (gaussian-kernel residual attention).

Reference computation (per batch sample b):
    sq_i   = ||x_i||^2
    G      = x @ x^T                      (T,T) gram
    sqdist = relu(sq_i + sq_j - 2 G)
    K      = exp(-sqdist * r + m) * (1 - eye)
    out    = x + K @ x

Algebraic restructuring (exact up to fp rounding):
    K_full = beta * e_i * e_j * exp(2 r g_ij),  e = exp(-r*sq), beta = exp(m)
    The diagonal of K_full is beta exactly, so zeroing it equals subtracting
    beta*x from K_full @ x:
    out = (1-beta)*x + beta * e ⊙_row ( E @ (e ⊙ x) ),  E = exp((2r x)^T x)

Sharding: data-parallel over batch B=16 across 8 NeuronCores (2 samples per
core, processed sequentially).

Measured TRN2 facts the structure exploits (see mb.py):
  - ACT exp is 1 elem/lane/cycle @1.2GHz and scale/bias APs cost ~270ns per
    instruction, so 2r is folded into the xTl gram operand and e into xs
    (the Y stationary operand): the exp instruction takes no arguments.
  - Matmuls in this dependency structure run at the PE's unramped clock
    (~427ns per N=512 fp32r matmul). Row-tiling via tile_position runs two
    K=64 matmuls concurrently on array row-groups 0/64 (~2x, measured
    90.8ns/MM in a clean stream), so the gram is computed in row-tiled
    pairs: G tile = [G_{2jp} chunk | G_{2jp+1} chunk] with x^T duplicated
    into both partition halves of xTl/xTr.
  - Matmul PSUM dst must start at partition 0 (col-tiling/dst-base-64
    crashes walrus codegen), so the Y matmul stays un-tiled (K=128, M=64).

Per-core dataflow (per sample):
    x_sb  (128,16,64)  natural layout, partition p = t%128, k = t//128
    xTr   (128,2048)   x^T dup'd in both halves (PE transpose + DVE evac+dup)
    xTl   = 2r * xTr   (one DVE pass per 512-col group)
    sq -> e=exp(-r*sq) -> f=beta*e, xs=e⊙x, ax=alpha*x
    two passes q over YT column halves (psum: G pingpong 4 banks + YT half
    2 banks + transpose scratch 2 banks = 8):
      16 steps st=(jp,c):
        G (128,1024) psum = 2 concurrent row-tiled K=64 matmuls (N=512)
        E (128,1024) sbuf = ACT exp(G), argument-free
        YTq[:, 512c:+512]  += xs[j0]-stationary @ E[:,0:512]
                           += xs[j1]-stationary @ E[:,512:1024]
      YTq -> sbuf -> 8 PE transposes -> DVE stt: out = ax + f ⊙_row Y ; DMA.
    Prep for the next sample and the out-phase of the previous pass are
    emitted interleaved into the step loops (filler drain) so no engine
    idles at sample/pass boundaries.
"""

import numpy as np

import concourse.bass as bass
import concourse.tile as tile
from concourse import bacc, mybir
from concourse.bass_utils import run_bass_kernel_spmd
from concourse.masks import make_identity

F32 = mybir.dt.float32
F32R = mybir.dt.float32r  # fp32 data, PE fast-fp32 matmul mode
AF = mybir.ActivationFunctionType
B, T, C = 16, 2048, 64
N_CORES = 8
BPC = B // N_CORES          # samples per core
NK = T // 128               # 16 row-blocks of 128


def build_nc(reps=1, stages='all'):
    nc = bacc.Bacc("TRN2", target_bir_lowering=False, debug=False, num_devices=N_CORES)
    x_in = nc.dram_tensor("x", [BPC, T, C], F32, kind="ExternalInput")
    r_in = nc.dram_tensor("r_sigma", [1], F32, kind="ExternalInput")
    m_in = nc.dram_tensor("margin", [1], F32, kind="ExternalInput")
    o_out = nc.dram_tensor("out", [BPC, T, C], F32, kind="ExternalOutput")

    with tile.TileContext(nc) as tc:
        if reps == 1:
            _body(tc, o_out.ap(), x_in.ap(), r_in.ap(), m_in.ap(), stages)
        else:
            with tc.For_i(0, reps, 1):
                _body(tc, o_out.ap(), x_in.ap(), r_in.ap(), m_in.ap(), stages)
    nc.compile()
    return nc


LEVELS = {'prep': 0, 'gram': 1, 'exp': 2, 'y': 3, 'all': 4}


def _body(tc, out_ap, x_ap, r_ap, m_ap, stages='all'):
    lvl = LEVELS[stages]
    nc = tc.nc
    with (
        tc.tile_pool(name="consts", bufs=1) as consts,
        tc.tile_pool(name="sx", bufs=2) as sx,
        tc.tile_pool(name="epool", bufs=3) as epool,
        tc.tile_pool(name="psG", bufs=2, space="PSUM") as psG,
        tc.tile_pool(name="psY", bufs=1, space="PSUM") as psY,
        tc.tile_pool(name="psT", bufs=2, space="PSUM") as psT,
    ):
        # ---- one-time constants ----
        ident = consts.tile([128, 128], F32)
        make_identity(nc, ident)
        rb = consts.tile([128, 1], F32)
        nc.gpsimd.dma_start(out=rb, in_=r_ap.to_broadcast((128, 1)))
        mb = consts.tile([128, 1], F32)
        nc.gpsimd.dma_start(out=mb, in_=m_ap.to_broadcast((128, 1)))
        negr = consts.tile([128, 1], F32)
        nc.vector.tensor_scalar_mul(out=negr, in0=rb, scalar1=-1.0)
        s2r = consts.tile([128, 1], F32)
        nc.vector.tensor_scalar_mul(out=s2r, in0=rb, scalar1=2.0)
        beta = consts.tile([128, 1], F32)
        nc.scalar.activation(out=beta, in_=mb, func=AF.Exp)
        alpha = consts.tile([128, 1], F32)  # 1 - beta
        nc.vector.tensor_scalar(
            out=alpha, in0=beta, scalar1=-1.0, scalar2=1.0,
            op0=mybir.AluOpType.mult, op1=mybir.AluOpType.add,
        )

        # prefetch both samples' inputs up front (per-ring DMA FIFOs run in
        # emission order; loads must not queue behind stores)
        x_sbs = []
        for s in range(BPC):
            xv = x_ap[s].rearrange("(p k) c -> p k c", p=128)
            x_sb = sx.tile([128, NK, C], F32, tag="x_sb", name=f"x_sb_{s}")
            nc.sync.dma_start(out=x_sb[:, 0:8, :], in_=xv[:, 0:8, :])
            nc.gpsimd.dma_start(out=x_sb[:, 8:NK, :], in_=xv[:, 8:NK, :])
            x_sbs.append(x_sb)

        # ---- per-sample prep, emitted as filler closures ----
        # returns (tiles dict, list of closures to emit)
        def make_prep(s):
            x_sb = x_sbs[s]
            d = {}
            # Gram operand tiles for row-tiled (K=64 x 2 concurrent) grams.
            # Both tiles carry x^T duplicated into partition halves 0-63 and
            # 64-127 so row-group 0 computes G_j while row-group 64 computes
            # G_{j+1} in the same pass. xTl is pre-scaled by 2r so the exp
            # needs no scale AP: G' = (2r x)^T x = 2r G.
            d['xTr'] = sx.tile([128, T], F32R, tag="xTr", name=f"xTr_{s}")
            d['xTl'] = sx.tile([128, T], F32R, tag="xTl", name=f"xTl_{s}")
            d['xs'] = sx.tile([128, NK, C], F32R, tag="xs", name=f"xs_{s}")
            d['sq'] = sx.tile([128, NK], F32, tag="sq", name=f"sq_{s}")
            d['ebias'] = sx.tile([128, NK], F32, tag="ebias", name=f"ebias_{s}")
            d['e'] = sx.tile([128, NK], F32, tag="e", name=f"e_{s}")
            d['f'] = sx.tile([128, NK], F32, tag="f", name=f"f_{s}")
            d['ax'] = sx.tile([128, NK, C], F32, tag="ax", name=f"ax_{s}")
            ops = []

            def xt_group(g):
                def emit():
                    xtr = psT.tile([64, 4, 128], F32, tag="T", name=f"xtr_{s}_{g}")
                    for kk in range(4):
                        k = 4 * g + kk
                        nc.tensor.transpose(
                            out=xtr[:, kk, :], in_=x_sb[:, k, :], identity=ident
                        )
                    cols = slice(512 * g, 512 * (g + 1))
                    nc.vector.tensor_copy(
                        out=d['xTr'][:64, cols],
                        in_=xtr.rearrange("p a b -> p (a b)"),
                    )
                    nc.vector.tensor_copy(
                        out=d['xTr'][64:128, cols], in_=d['xTr'][:64, cols],
                    )
                    nc.vector.tensor_scalar_mul(
                        out=d['xTl'][:, cols], in0=d['xTr'][:, cols], scalar1=s2r,
                    )
                return emit
            for g in range(4):
                ops.append(xt_group(g))

            def scalars():
                xsq = sx.tile([128, NK, C], F32, tag="xsq", name=f"xsq_{s}")
                nc.vector.tensor_mul(xsq, x_sb, x_sb)
                nc.vector.reduce_sum(out=d['sq'], in_=xsq, axis=mybir.AxisListType.X)
                nc.vector.tensor_scalar_mul(out=d['ebias'], in0=d['sq'], scalar1=negr)
                nc.scalar.activation(out=d['e'], in_=d['ebias'], func=AF.Exp)
                nc.vector.tensor_scalar_mul(out=d['f'], in0=d['e'], scalar1=beta)
            ops.append(scalars)

            def xsop(half):
                def emit():
                    for k in range(8 * half, 8 * half + 8):
                        nc.vector.tensor_scalar_mul(
                            out=d['xs'][:, k, :], in0=x_sb[:, k, :],
                            scalar1=d['e'][:, k: k + 1],
                        )
                return emit
            ops.append(xsop(0))
            ops.append(xsop(1))

            def axop():
                nc.vector.tensor_scalar_mul(out=d['ax'], in0=x_sbs[s], scalar1=alpha)
            ops.append(axop)
            return d, ops

        # ---- out-phase for one YT column half, as filler closures ----
        def make_out(s, q, YTq, prep):
            ov = out_ap[s].rearrange("(p k) c -> p k c", p=128)
            YTsb = sx.tile([64, 1024], F32, tag="YTsb", name=f"YTsb_{s}_{q}")
            ops = []

            def evac(h):
                def emit():
                    nc.vector.tensor_copy(
                        out=YTsb[:, 512 * h: 512 * (h + 1)],
                        in_=YTq[:, 512 * h: 512 * (h + 1)],
                    )
                return emit
            ops.append(evac(0))
            ops.append(evac(1))

            dma_legs = [nc.scalar, nc.sync]

            def outgroup(g):
                # g in 0,1 : blocks k = 8q + 4g + (0..3)
                def emit():
                    ytr = psT.tile([128, 4, C], F32, tag="T", name=f"ytr_{s}_{q}_{g}")
                    for kk in range(4):
                        nc.tensor.transpose(
                            out=ytr[:, kk, :],
                            in_=YTsb[:, 128 * (4 * g + kk): 128 * (4 * g + kk + 1)],
                            identity=ident[:64, :64],
                        )
                    outsb = sx.tile([128, 4, C], F32, tag="outsb",
                                    name=f"outsb_{s}_{q}_{g}")
                    for kk in range(4):
                        k = 8 * q + 4 * g + kk
                        nc.vector.scalar_tensor_tensor(
                            out=outsb[:, kk, :], in0=ytr[:, kk, :],
                            scalar=prep['f'][:, k: k + 1],
                            in1=prep['ax'][:, k, :], op0=mybir.AluOpType.mult,
                            op1=mybir.AluOpType.add,
                        )
                    dma_legs[g].dma_start(
                        out=ov[:, 8 * q + 4 * g: 8 * q + 4 * g + 4, :], in_=outsb
                    )
                return emit
            ops.append(outgroup(0))
            ops.append(outgroup(1))
            return ops

        # ---- main: sequential samples, 2 YT column passes each ----
        filler = []   # deque of closures to interleave into j-loops

        def drain(n):
            for _ in range(min(n, len(filler))):
                filler.pop(0)()

        prep0, ops0 = make_prep(0)
        for op in ops0:
            op()          # first sample prep emitted immediately
        preps = {0: prep0}

        for s in range(BPC):
            prep = preps[s]
            xTl, xTr = prep['xTl'], prep['xTr']
            x_sb = x_sbs[s]
            if s + 1 < BPC:
                preps[s + 1], nops = make_prep(s + 1)
                filler.extend(nops)

            for q in range(2):
                YTq = psY.tile([64, 1024], F32, tag="YT", name=f"YT_{s}_{q}")
                cbase = 1024 * q

                # step st = (jpair, c): row-group 0 computes G_{2jp}[:, c-chunk]
                # while row-group 64 concurrently computes G_{2jp+1}[:, c-chunk]
                def emit_gram(st):
                    jp, c = st >> 1, st & 1
                    j0, j1 = 2 * jp, 2 * jp + 1
                    n0 = cbase + 512 * c
                    G = psG.tile([128, 1024], F32, tag="G", name=f"G_{s}_{q}_{st}")
                    nc.tensor.matmul(
                        out=G[:, 0:512],
                        lhsT=xTl[0:64, 128 * j0: 128 * (j0 + 1)],
                        rhs=xTr[0:64, n0: n0 + 512],
                        start=True, stop=True, tile_position=(0, 0),
                    )
                    nc.tensor.matmul(
                        out=G[:, 512:1024],
                        lhsT=xTl[64:128, 128 * j1: 128 * (j1 + 1)],
                        rhs=xTr[64:128, n0: n0 + 512],
                        start=True, stop=True, tile_position=(64, 0),
                    )
                    return G

                def emit_exp(G, st):
                    E = epool.tile([128, 1024], F32R, tag="E", name=f"E_{s}_{q}_{st}")
                    nc.scalar.activation(out=E, in_=G, func=AF.Exp)
                    return E

                def emit_y(E, st):
                    jp, c = st >> 1, st & 1
                    j0, j1 = 2 * jp, 2 * jp + 1
                    yslice = YTq[:, 512 * c: 512 * (c + 1)]
                    nc.tensor.matmul(
                        out=yslice, lhsT=prep['xs'][:, j0, :], rhs=E[:, 0:512],
                        start=(jp == 0), stop=False,
                    )
                    nc.tensor.matmul(
                        out=yslice, lhsT=prep['xs'][:, j1, :], rhs=E[:, 512:1024],
                        start=False, stop=(jp == 7),
                    )

                if lvl >= 1:
                    G = emit_gram(0)
                    for st in range(NK):
                        E = emit_exp(G, st) if lvl >= 2 else None
                        if st + 1 < NK:
                            G = emit_gram(st + 1)
                        if lvl >= 3:
                            emit_y(E, st)
                        drain(1)

                if lvl >= 4:
                    filler.extend(make_out(s, q, YTq, prep))

        # drain any remaining filler (last pass's out-phase)
        drain(len(filler) + 1)


_NC_CACHE = {}


def _get_nc(reps=1, stages='all'):
    key = (reps, stages)
    if key not in _NC_CACHE:
        _NC_CACHE[key] = build_nc(reps, stages)
    return _NC_CACHE[key]


def _run(x, r_sigma, margin, trace=False, reps=1, stages='all'):
    nc = _get_nc(reps, stages)
    x = np.ascontiguousarray(np.asarray(x, dtype=np.float32))
    r_sigma = np.ascontiguousarray(np.asarray(r_sigma, dtype=np.float32))
    margin = np.ascontiguousarray(np.asarray(margin, dtype=np.float32))
    in_maps = [
        {
            "x": np.ascontiguousarray(x[c * BPC: (c + 1) * BPC]),
            "r_sigma": r_sigma,
            "margin": margin,
        }
        for c in range(N_CORES)
    ]
    res = run_bass_kernel_spmd(nc, in_maps, core_ids=list(range(N_CORES)), trace=trace)
    out = np.concatenate([res.results[c]["out"] for c in range(N_CORES)], axis=0)
    return out, res


def kernel(x, r_sigma, margin):
    out, _ = _run(x, r_sigma, margin, trace=False)
    return out
